# revision 12
# baseline (speedup 1.0000x reference)
"""Trainium2 Bass kernel for modulated deformable conv2d (torchvision semantics).

Problem (hardcoded): input [4,64,128,128] f32, offset [4,18,128,128], mask
[4,9,128,128], weight [64,64,3,3], bias [64]; stride 1, pad 1, dil 1.

Strategy (8 NeuronCores, SPMD, no collectives):
  - Shard: core = (sample b, row-half h).  Each core computes output rows
    [h*64, h*64+64) of sample b => out slice [64, 8192] f32.
  - Bilinear gather via 4 corner planes (bf16): each sample point is a
    single 512-byte-row NON-TRANSPOSE dma_gather from a DRAM table, issued
    round-robin on 4 SWDGE queues (engages all 8 Q7 cores; measured
    2.2ns/idx vs 8ns on one queue).  Row = [P00(c0:64) P01 P10 P11].
    Data lands sample-major: partition = sample%128, free = (slot, 256).
  - Per (block t of 512 px, pixel-chunk pc of 128 px): one gather call of
    1152 idx = 9 taps x 128 px (partition = pixel, slot = tap).
    ACT expands per-sample corner weights [128,9,4] -> [128,9,4,64] bf16
    (stride-0 broadcast read), DVE multiplies and pair-sums the 4 corner
    planes -> val [128 px, 9 taps, 64 ch].
  - PE transposes val in tap-pairs ([128,128] via identity, is_transpose)
    into a PSUM strip [128, 5*512] (tap-pair-major, pixel-chunk columns);
    DVE copies the strip to SBUF; conv runs as 5 accumulated matmuls per
    block (contraction = 2 taps x 64 ch) into ops [64, 512].
  - Host precomputes gather indices and corner weights; device replicates
    the 16-wrap indices across Q7 cores.
"""

import sys

if "/opt/trn_rl_repo" not in sys.path:
    sys.path.insert(0, "/opt/trn_rl_repo")

import numpy as np
import ml_dtypes

BF16 = ml_dtypes.bfloat16

# problem dims
B, C, H, W = 4, 64, 128, 128
O, K = 64, 9
PAD = 8                     # gather window margin (|offset| <= ~6.8 required)
TG = H + 2 * PAD + 1        # 145: table grid covers y,x in [-PAD, H+PAD]
GEXT = TG + 1               # 146: extended image grid (corner planes read +1)
NROWS = TG * TG             # 21025
NROWS_PAD = ((NROWS + 127) // 128) * 128   # 21120
NPIX = H * W // 2           # 8192 output pixels per core
NBLK = 16                   # pixel blocks per core
BLK = NPIX // NBLK          # 512 pixels per block
NPC = 4                     # pixel chunks per block
PCW = BLK // NPC            # 128 pixels per chunk
GCALL = 2 * K * PCW         # 2304 gather indices per call (2 pixel chunks)
NSAMP = K * NPIX            # 73728 sample points per core
KP = 5                      # tap pairs (4 full + 1 single)

L2F = NSAMP // 16           # 4608 idx slots per 16-partition wrap

_CACHE = {}


def _split_excess_waits(nc, limit=1):
    """Walrus in this image caps sync-wait commands per instruction; hoist
    excess waits onto preceding same-engine NoOps (engine streams are
    in-order, so blocking earlier on a prefix of the waits is equivalent)."""
    from concourse import mybir

    n = 0
    for fn in nc.m.functions:
        for blk in fn.blocks:
            new = []
            for inst in blk.instructions:
                si = inst.sync_info
                if si is not None and len(si.on_wait) > limit:
                    waits = list(si.on_wait)
                    head, keep = waits[:-limit], waits[-limit:]
                    for i in range(0, len(head), limit):
                        n += 1
                        new.append(mybir.InstNoOp(
                            name=f"waitsplit_{n}",
                            sync_info=mybir.SyncInfo(
                                on_wait=head[i:i + limit], on_update=[]),
                            bass_nofuse=True,
                            engine=inst.engine,
                        ))
                    inst.sync_info = mybir.SyncInfo(
                        on_wait=keep, on_update=list(si.on_update))
                new.append(inst)
            blk.instructions = new


def _build_program():
    import concourse.bass as bass
    import concourse.tile as tile
    from concourse import mybir

    f32 = mybir.dt.float32
    bf16 = mybir.dt.bfloat16
    i16 = mybir.dt.int16

    nc = bass.Bass("TRN2", target_bir_lowering=False, debug=False,
                   enable_asserts=False, dynamic_dma_scratch_size=65536,
                   num_swdge_queues=4)

    tab_d = nc.dram_tensor("tab", [NROWS_PAD, 4 * C], bf16, kind="ExternalInput")
    idx_d = nc.dram_tensor("idx", [16, L2F], i16, kind="ExternalInput")
    bw_d = nc.dram_tensor("bw", [128, NBLK, NPC, K, 4], bf16,
                          kind="ExternalInput")
    wt_d = nc.dram_tensor("wt", [128, KP, O], bf16, kind="ExternalInput")
    idn_d = nc.dram_tensor("idn", [128, 128], bf16, kind="ExternalInput")
    bias_d = nc.dram_tensor("bias", [O, 1], f32, kind="ExternalInput")
    out_d = nc.dram_tensor("out", [O, NPIX], f32, kind="ExternalOutput")

    from concourse import library_config

    with tile.TileContext(nc) as tc:
        nc.gpsimd.load_library(library_config.mlp)   # provides DMAGatherAnt
        with tc.tile_pool(name="const", bufs=1) as cp:
            idxr = cp.tile([128, L2F], i16, tag="idxr")
            _eng = [nc.sync, nc.scalar]
            IDX0 = 288                     # first two 2304-idx calls
            for grp in range(8):
                _eng[grp % 2].dma_start(
                    idxr[16 * grp:16 * (grp + 1), 0:IDX0],
                    idx_d.ap()[:, 0:IDX0])
            for grp in range(8):
                _eng[grp % 2].dma_start(
                    idxr[16 * grp:16 * (grp + 1), IDX0:],
                    idx_d.ap()[:, IDX0:])

            w_sb = cp.tile([128, KP, O], bf16, tag="wsb")
            nc.scalar.dma_start(w_sb[:], wt_d.ap())
            bias_sb = cp.tile([O, 1], f32, tag="bias")
            nc.scalar.dma_start(bias_sb[:], bias_d.ap())
            ident = cp.tile([128, 128], bf16, tag="ident")
            nc.scalar.dma_start(ident[:], idn_d.ap())
            bwsb = cp.tile([128, NBLK, NPC, K, 4], bf16, tag="bw")
            nc.scalar.dma_start(bwsb[:], bw_d.ap())

            rg = nc.gpsimd.to_reg(GCALL)

            with (
                tc.tile_pool(name="g", bufs=3) as gp,
                tc.tile_pool(name="bx", bufs=3) as bxp,
                tc.tile_pool(name="tm", bufs=3) as tmp_,
                tc.tile_pool(name="val", bufs=3) as vp,
                tc.tile_pool(name="trp", bufs=2, space="PSUM") as trpp,
                tc.tile_pool(name="opsum", bufs=2, space="PSUM") as opp,
                tc.tile_pool(name="vsb", bufs=2) as vsbp,
                tc.tile_pool(name="ob", bufs=2) as obp,
            ):
                gtiles = {}

                def get_call(t, h):
                    key = t * 2 + h
                    if key not in gtiles:
                        g = gp.tile([128, 2, K, 4 * C], bf16, tag=f"g{h}")
                        s0 = key * (GCALL // 16)
                        nc.gpsimd.dma_gather(
                            g[:].rearrange("p l k e -> p (l k) e"),
                            tab_d.ap(), idxr[:, s0:s0 + GCALL // 16],
                            GCALL, rg, 4 * C,
                            transpose=False, single_packet=False,
                            queue_num=key % 4)
                        gtiles[key] = g
                    return gtiles[key]

                for t in range(NBLK):
                    # prefetch next block's gathers onto the queues
                    if t + 1 < NBLK:
                        for h in range(2):
                            get_call(t + 1, h)

                    tr = trpp.tile([128, KP, BLK], bf16, tag="tr")
                    for h in range(2):
                        g = get_call(t, h)
                        # beta expansion: [128, 2, K, 4] -> [128, 2, K, 4, 64]
                        bx = bxp.tile([128, 2, K, 4, C], bf16, tag="bx")
                        bsrc = (bwsb[:, t, 2 * h:2 * h + 2, :, :].unsqueeze(4)
                                .broadcast_to([128, 2, K, 4, C]))
                        nc.scalar.copy(bx[:], bsrc)
                        # weighted corners, then pair-sum 4 planes -> val
                        tm = tmp_.tile([128, 2, K, 4, C], bf16, tag="tm")
                        nc.vector.tensor_mul(
                            tm[:].rearrange("p l k q c -> p (l k q c)"),
                            g[:].rearrange("p l k e -> p (l k e)"),
                            bx[:].rearrange("p l k q c -> p (l k q c)"))
                        u = vp.tile([128, 2, K, 2, C], bf16, tag="u")
                        val = vp.tile([128, 2, K, C], bf16, tag="val")
                        nc.vector.tensor_add(u[:], tm[:, :, :, 0:2, :],
                                             tm[:, :, :, 2:4, :])
                        nc.vector.tensor_add(val[:], u[:, :, :, 0, :],
                                             u[:, :, :, 1, :])
                        # transpose tap-pairs into the PSUM strip
                        for pcl in range(2):
                            pc = 2 * h + pcl
                            vf = val[:, pcl].rearrange("p k c -> p (k c)")
                            for kp in range(4):
                                nc.tensor.matmul(
                                    tr[:, kp, pc * PCW:(pc + 1) * PCW],
                                    vf[:, 128 * kp:128 * (kp + 1)],
                                    ident[:], is_transpose=True)
                            nc.tensor.matmul(
                                tr[0:64, 4, pc * PCW:(pc + 1) * PCW],
                                vf[:, 512:576], ident[:], is_transpose=True)

                    vsb = vsbp.tile([128, KP, BLK], bf16, tag="vsb")
                    nc.vector.tensor_copy(
                        vsb[:, 0:4].rearrange("p k b -> p (k b)"),
                        tr[:, 0:4].rearrange("p k b -> p (k b)"))
                    nc.vector.tensor_copy(vsb[0:64, 4], tr[0:64, 4])

                    ops = opp.tile([O, BLK], f32, tag="ops")
                    for kp in range(4):
                        nc.tensor.matmul(ops[:], w_sb[:, kp, :], vsb[:, kp, :],
                                         start=(kp == 0), stop=False)
                    nc.tensor.matmul(ops[:], w_sb[0:64, 4, :], vsb[0:64, 4, :],
                                     start=False, stop=True)

                    ob = obp.tile([O, BLK], f32, tag="ob")
                    nc.scalar.add(ob[:], ops[:], bias_sb[:, 0:1])
                    nc.sync.dma_start(out_d.ap()[:, t * BLK:(t + 1) * BLK], ob[:])

    _split_excess_waits(nc)
    from concourse.library_overlay import lower_extended_insts
    lower_extended_insts(nc)
    return nc


def _host_prep(input, offset, mask, weight, bias):
    x = np.asarray(input, np.float32)
    off = np.asarray(offset, np.float32)
    msk = np.asarray(mask, np.float32)
    w = np.asarray(weight, np.float32)
    b = np.asarray(bias, np.float32)

    amax = float(np.abs(off).max())
    if amax >= PAD - 1.2:
        raise ValueError(f"offset magnitude {amax} exceeds supported window")

    f32 = np.float32

    # per-sample gather tables; row = 4 corner planes x 64 ch (plane-major)
    tabs = []
    for bb in range(B):
        E = np.zeros((C, GEXT, GEXT), f32)
        E[:, PAD:PAD + H, PAD:PAD + W] = x[bb]
        E8 = E.astype(BF16)
        P00 = E8[:, :TG, :TG]
        P01 = E8[:, :TG, 1:TG + 1]
        P10 = E8[:, 1:TG + 1, :TG]
        P11 = E8[:, 1:TG + 1, 1:TG + 1]
        planes = np.stack([P00, P01, P10, P11], 0)   # [4, C, TG, TG]
        arr = planes.transpose(2, 3, 0, 1)           # [TG, TG, 4, C]
        rows = np.zeros((NROWS_PAD, 4 * C), BF16)
        rows[:NROWS] = arr.reshape(NROWS, 4 * C)
        tabs.append(rows)

    # conv lhsT per tap-pair: partition p<64 -> (tap 2kp, ch p),
    # p>=64 -> (tap 2kp+1, ch p-64); kp=4 single tap 8 on rows 0:64.
    wk = w.reshape(O, C, K)                      # [O, C, K]
    wt = np.zeros((128, KP, O), f32)
    for kp in range(4):
        wt[0:64, kp, :] = wk[:, :, 2 * kp].T
        wt[64:128, kp, :] = wk[:, :, 2 * kp + 1].T
    wt[0:64, 4, :] = wk[:, :, 8].T

    bias2 = np.ascontiguousarray(b.reshape(O, 1))
    idn = np.eye(128, dtype=BF16)

    karr = np.arange(K)
    p = np.arange(NPIX)
    ylo = p // W
    xloc = p % W

    def tojd(a):  # [K, NPIX] -> flat j order (t, pc, k, px)
        return np.ascontiguousarray(
            a.reshape(K, NBLK, NPC, PCW).transpose(1, 2, 0, 3).reshape(-1))

    in_maps = []
    for core in range(8):
        bb, h = divmod(core, 2)
        yg = h * 64 + ylo                                   # [NPIX] global y
        offv = off[bb].reshape(K, 2, H, W)
        oy_kp = offv[:, 0][:, yg, xloc]                     # [K, NPIX]
        ox_kp = offv[:, 1][:, yg, xloc]
        m_kp = msk[bb][:, yg, xloc]
        by = yg[None, :] - 1 + (karr // 3)[:, None]
        bx = xloc[None, :] - 1 + (karr % 3)[:, None]

        y0 = np.floor(oy_kp)
        x0 = np.floor(ox_kp)
        ly = oy_kp - y0
        lx = ox_kp - x0
        idx_kp = (y0 + by + PAD) * TG + (x0 + bx + PAD)     # [K, NPIX]

        idxj = tojd(idx_kp).astype(np.int64)
        assert idxj.min() >= 0 and idxj.max() < NROWS
        idx16 = np.ascontiguousarray(
            idxj.reshape(NSAMP // 16, 16).T).astype(np.int16)

        mj = tojd(m_kp).astype(f32)
        lxj = tojd(lx).astype(f32)
        lyj = tojd(ly).astype(f32)
        # corner weights matching planes P00..P11, [4, NSAMP]
        bws = np.stack([mj * (1 - lyj) * (1 - lxj),
                        mj * (1 - lyj) * lxj,
                        mj * lyj * (1 - lxj),
                        mj * lyj * lxj], 0)
        # -> [128 px, NBLK, NPC, K, 4]
        bw_host = np.ascontiguousarray(
            bws.reshape(4, NBLK, NPC, K, PCW).transpose(4, 1, 2, 3, 0)
        ).astype(BF16)

        in_maps.append({
            "tab": tabs[bb],
            "idx": idx16,
            "bw": bw_host,
            "wt": wt.astype(BF16),
            "idn": idn,
            "bias": bias2,
        })
    return in_maps


def _install_ntff_shim():
    """Provide antenv.axon_hooks (missing in this image) so trace=True works."""
    import types
    if "antenv.axon_hooks" in sys.modules:
        return
    sys.path.insert(0, "/root/.axon_site")
    from trn_agent_boot.trn_boot import _ntff_profile_via_ctypes
    hook = _ntff_profile_via_ctypes("/opt/axon/libaxon_pjrt.so")
    mod = types.ModuleType("antenv.axon_hooks")
    mod.get_axon_ntff_profile_hook = lambda: hook
    mod.set_axon_ntff_profile_hook = lambda h: None
    sys.modules["antenv.axon_hooks"] = mod


def kernel(input, offset, mask, weight, bias, _trace=False):
    if _trace:
        _install_ntff_shim()
    from concourse.bass_utils import run_bass_kernel_spmd

    if "nc" not in _CACHE:
        _CACHE["nc"] = _build_program()
    nc = _CACHE["nc"]

    in_maps = _host_prep(input, offset, mask, weight, bias)
    res = run_bass_kernel_spmd(
        nc, in_maps, core_ids=list(range(8)),
        trace=_trace,
        trace_cores=list(range(8)) if _trace else None,
    )
    kernel.last_results = res

    out = np.empty((B, O, H, W), np.float32)
    for core in range(8):
        bb, h = divmod(core, 2)
        blockout = res.results[core]["out"]       # [64, 8192] f32
        out[bb, :, h * 64:(h + 1) * 64, :] = blockout.reshape(O, 64, W)
    return out


# revision 13
# speedup vs baseline: 1.2134x; 1.2134x over previous
"""Trainium2 Bass kernel for modulated deformable conv2d (torchvision semantics).

Problem (hardcoded): input [4,64,128,128] f32, offset [4,18,128,128], mask
[4,9,128,128], weight [64,64,3,3], bias [64]; stride 1, pad 1, dil 1.

Strategy (8 NeuronCores, SPMD, no collectives):
  - Shard: core = (sample b, row-half h).  Each core computes output rows
    [h*64, h*64+64) of sample b => out slice [64, 8192] f32.
  - Bilinear gather via 4 corner planes (bf16): each sample point is a
    single 512-byte-row NON-TRANSPOSE dma_gather from a DRAM table, issued
    round-robin on 4 SWDGE queues (engages all 8 Q7 cores; measured
    2.2ns/idx vs 8ns on one queue).  Row = [P00(c0:64) P01 P10 P11].
    Data lands sample-major: partition = sample%128, free = (slot, 256).
  - Per (block t of 512 px, pixel-chunk pc of 128 px): one gather call of
    1152 idx = 9 taps x 128 px (partition = pixel, slot = tap).
    ACT expands per-sample corner weights [128,9,4] -> [128,9,4,64] bf16
    (stride-0 broadcast read), DVE multiplies and pair-sums the 4 corner
    planes -> val [128 px, 9 taps, 64 ch].
  - PE transposes val in tap-pairs ([128,128] via identity, is_transpose)
    into a PSUM strip [128, 5*512] (tap-pair-major, pixel-chunk columns);
    DVE copies the strip to SBUF; conv runs as 5 accumulated matmuls per
    block (contraction = 2 taps x 64 ch) into ops [64, 512].
  - Host precomputes gather indices and corner weights; device replicates
    the 16-wrap indices across Q7 cores.
"""

import sys

if "/opt/trn_rl_repo" not in sys.path:
    sys.path.insert(0, "/opt/trn_rl_repo")

import numpy as np
import ml_dtypes

BF16 = ml_dtypes.bfloat16

# problem dims
B, C, H, W = 4, 64, 128, 128
O, K = 64, 9
PAD = 8                     # gather window margin (|offset| <= ~6.8 required)
TG = H + 2 * PAD + 1        # 145: table grid covers y,x in [-PAD, H+PAD]
GEXT = TG + 1               # 146: extended image grid (corner planes read +1)
NROWS = TG * TG             # 21025
NROWS_PAD = ((NROWS + 127) // 128) * 128   # 21120
NPIX = H * W // 2           # 8192 output pixels per core
NBLK = 16                   # pixel blocks per core
BLK = NPIX // NBLK          # 512 pixels per block
NPC = 4                     # pixel chunks per block
PCW = BLK // NPC            # 128 pixels per chunk
GCALL = K * PCW             # 1152 gather indices per call
NSAMP = K * NPIX            # 73728 sample points per core
KP = 5                      # tap pairs (4 full + 1 single)

L2F = NSAMP // 16           # 4608 idx slots per 16-partition wrap

_CACHE = {}


def _split_excess_waits(nc, limit=1):
    """Walrus in this image caps sync-wait commands per instruction; hoist
    excess waits onto preceding same-engine NoOps (engine streams are
    in-order, so blocking earlier on a prefix of the waits is equivalent)."""
    from concourse import mybir

    n = 0
    for fn in nc.m.functions:
        for blk in fn.blocks:
            new = []
            for inst in blk.instructions:
                si = inst.sync_info
                if si is not None and len(si.on_wait) > limit:
                    waits = list(si.on_wait)
                    head, keep = waits[:-limit], waits[-limit:]
                    for i in range(0, len(head), limit):
                        n += 1
                        new.append(mybir.InstNoOp(
                            name=f"waitsplit_{n}",
                            sync_info=mybir.SyncInfo(
                                on_wait=head[i:i + limit], on_update=[]),
                            bass_nofuse=True,
                            engine=inst.engine,
                        ))
                    inst.sync_info = mybir.SyncInfo(
                        on_wait=keep, on_update=list(si.on_update))
                new.append(inst)
            blk.instructions = new


def _build_program():
    import concourse.bass as bass
    import concourse.tile as tile
    from concourse import mybir

    f32 = mybir.dt.float32
    bf16 = mybir.dt.bfloat16
    i16 = mybir.dt.int16

    nc = bass.Bass("TRN2", target_bir_lowering=False, debug=False,
                   enable_asserts=False, dynamic_dma_scratch_size=65536,
                   num_swdge_queues=4)

    tab_d = nc.dram_tensor("tab", [NROWS_PAD, 4 * C], bf16, kind="ExternalInput")
    idx_d = nc.dram_tensor("idx", [16, L2F], i16, kind="ExternalInput")
    bw_d = nc.dram_tensor("bw", [128, NBLK, NPC, K, 4], bf16,
                          kind="ExternalInput")
    wt_d = nc.dram_tensor("wt", [128, KP, O], bf16, kind="ExternalInput")
    idn_d = nc.dram_tensor("idn", [128, 128], bf16, kind="ExternalInput")
    bias_d = nc.dram_tensor("bias", [O, 1], f32, kind="ExternalInput")
    out_d = nc.dram_tensor("out", [O, NPIX], f32, kind="ExternalOutput")

    from concourse import library_config

    with tile.TileContext(nc) as tc:
        nc.gpsimd.load_library(library_config.mlp)   # provides DMAGatherAnt
        with tc.tile_pool(name="const", bufs=1) as cp:
            idxr = cp.tile([128, L2F], i16, tag="idxr")
            _eng = [nc.sync, nc.scalar]
            IDX0 = 288                     # first four 1152-idx calls
            for grp in range(8):
                _eng[grp % 2].dma_start(
                    idxr[16 * grp:16 * (grp + 1), 0:IDX0],
                    idx_d.ap()[:, 0:IDX0])
            for grp in range(8):
                _eng[grp % 2].dma_start(
                    idxr[16 * grp:16 * (grp + 1), IDX0:],
                    idx_d.ap()[:, IDX0:])

            w_sb = cp.tile([128, KP, O], bf16, tag="wsb")
            nc.scalar.dma_start(w_sb[:], wt_d.ap())
            bias_sb = cp.tile([O, 1], f32, tag="bias")
            nc.scalar.dma_start(bias_sb[:], bias_d.ap())
            ident = cp.tile([128, 128], bf16, tag="ident")
            nc.scalar.dma_start(ident[:], idn_d.ap())
            bwsb = cp.tile([128, NBLK, NPC, K, 4], bf16, tag="bw")
            nc.scalar.dma_start(bwsb[:], bw_d.ap())

            rg = nc.gpsimd.to_reg(GCALL)

            with (
                tc.tile_pool(name="g", bufs=3) as gp,
                tc.tile_pool(name="bx", bufs=3) as bxp,
                tc.tile_pool(name="tm", bufs=3) as tmp_,
                tc.tile_pool(name="val", bufs=3) as vp,
                tc.tile_pool(name="trp", bufs=2, space="PSUM") as trpp,
                tc.tile_pool(name="opsum", bufs=2, space="PSUM") as opp,
                tc.tile_pool(name="vsb", bufs=2) as vsbp,
                tc.tile_pool(name="ob", bufs=2) as obp,
            ):
                gtiles = {}

                def get_call(t, pc):
                    key = t * NPC + pc
                    if key not in gtiles:
                        g = gp.tile([128, K, 4 * C], bf16, tag=f"g{pc}")
                        s0 = key * (GCALL // 16)
                        nc.gpsimd.dma_gather(
                            g[:], tab_d.ap(), idxr[:, s0:s0 + GCALL // 16],
                            GCALL, rg, 4 * C,
                            transpose=False, single_packet=False,
                            queue_num=key % 4)
                        gtiles[key] = g
                    return gtiles[key]

                for t in range(NBLK):
                    # prefetch next block's gathers onto the queues
                    if t + 1 < NBLK:
                        for pc in range(NPC):
                            get_call(t + 1, pc)

                    tr = trpp.tile([128, KP, BLK], bf16, tag="tr")
                    for pc in range(NPC):
                        g = get_call(t, pc)
                        # beta expansion: [128, K, 4] -> [128, K, 4, 64]
                        bx = bxp.tile([128, K, 4, C], bf16, tag="bx")
                        bsrc = (bwsb[:, t, pc, :, :].unsqueeze(3)
                                .broadcast_to([128, K, 4, C]))
                        nc.scalar.copy(bx[:], bsrc)
                        # weighted corners, then pair-sum 4 planes -> val
                        tm = tmp_.tile([128, K, 4, C], bf16, tag="tm")
                        nc.vector.tensor_mul(
                            tm[:].rearrange("p k q c -> p (k q c)"),
                            g[:].rearrange("p k e -> p (k e)"),
                            bx[:].rearrange("p k q c -> p (k q c)"))
                        u = vp.tile([128, K, 2, C], bf16, tag="u")
                        val = vp.tile([128, K, C], bf16, tag="val")
                        nc.vector.tensor_add(u[:], tm[:, :, 0:2, :],
                                             tm[:, :, 2:4, :])
                        nc.vector.tensor_add(val[:], u[:, :, 0, :], u[:, :, 1, :])
                        # transpose tap-pairs into the PSUM strip
                        vf = val[:].rearrange("p k c -> p (k c)")
                        for kp in range(4):
                            nc.tensor.matmul(
                                tr[:, kp, pc * PCW:(pc + 1) * PCW],
                                vf[:, 128 * kp:128 * (kp + 1)],
                                ident[:], is_transpose=True)
                        nc.tensor.matmul(
                            tr[0:64, 4, pc * PCW:(pc + 1) * PCW],
                            vf[:, 512:576], ident[:], is_transpose=True)

                    vsb = vsbp.tile([128, KP, BLK], bf16, tag="vsb")
                    nc.vector.tensor_copy(
                        vsb[:, 0:4].rearrange("p k b -> p (k b)"),
                        tr[:, 0:4].rearrange("p k b -> p (k b)"))
                    nc.vector.tensor_copy(vsb[0:64, 4], tr[0:64, 4])

                    ops = opp.tile([O, BLK], f32, tag="ops")
                    for kp in range(4):
                        nc.tensor.matmul(ops[:], w_sb[:, kp, :], vsb[:, kp, :],
                                         start=(kp == 0), stop=False)
                    nc.tensor.matmul(ops[:], w_sb[0:64, 4, :], vsb[0:64, 4, :],
                                     start=False, stop=True)

                    ob = obp.tile([O, BLK], f32, tag="ob")
                    nc.scalar.add(ob[:], ops[:], bias_sb[:, 0:1])
                    nc.sync.dma_start(out_d.ap()[:, t * BLK:(t + 1) * BLK], ob[:])

    _split_excess_waits(nc)
    from concourse.library_overlay import lower_extended_insts
    lower_extended_insts(nc)
    return nc


def _host_prep(input, offset, mask, weight, bias):
    x = np.asarray(input, np.float32)
    off = np.asarray(offset, np.float32)
    msk = np.asarray(mask, np.float32)
    w = np.asarray(weight, np.float32)
    b = np.asarray(bias, np.float32)

    amax = float(np.abs(off).max())
    if amax >= PAD - 1.2:
        raise ValueError(f"offset magnitude {amax} exceeds supported window")

    f32 = np.float32

    # per-sample gather tables; row = 4 corner planes x 64 ch (plane-major)
    tabs = []
    for bb in range(B):
        E = np.zeros((C, GEXT, GEXT), f32)
        E[:, PAD:PAD + H, PAD:PAD + W] = x[bb]
        E8 = E.astype(BF16)
        P00 = E8[:, :TG, :TG]
        P01 = E8[:, :TG, 1:TG + 1]
        P10 = E8[:, 1:TG + 1, :TG]
        P11 = E8[:, 1:TG + 1, 1:TG + 1]
        planes = np.stack([P00, P01, P10, P11], 0)   # [4, C, TG, TG]
        arr = planes.transpose(2, 3, 0, 1)           # [TG, TG, 4, C]
        rows = np.zeros((NROWS_PAD, 4 * C), BF16)
        rows[:NROWS] = arr.reshape(NROWS, 4 * C)
        tabs.append(rows)

    # conv lhsT per tap-pair: partition p<64 -> (tap 2kp, ch p),
    # p>=64 -> (tap 2kp+1, ch p-64); kp=4 single tap 8 on rows 0:64.
    wk = w.reshape(O, C, K)                      # [O, C, K]
    wt = np.zeros((128, KP, O), f32)
    for kp in range(4):
        wt[0:64, kp, :] = wk[:, :, 2 * kp].T
        wt[64:128, kp, :] = wk[:, :, 2 * kp + 1].T
    wt[0:64, 4, :] = wk[:, :, 8].T

    bias2 = np.ascontiguousarray(b.reshape(O, 1))
    idn = np.eye(128, dtype=BF16)

    karr = np.arange(K)
    p = np.arange(NPIX)
    ylo = p // W
    xloc = p % W

    def tojd(a):  # [K, NPIX] -> flat j order (t, pc, k, px)
        return np.ascontiguousarray(
            a.reshape(K, NBLK, NPC, PCW).transpose(1, 2, 0, 3).reshape(-1))

    in_maps = []
    for core in range(8):
        bb, h = divmod(core, 2)
        yg = h * 64 + ylo                                   # [NPIX] global y
        offv = off[bb].reshape(K, 2, H, W)
        oy_kp = offv[:, 0][:, yg, xloc]                     # [K, NPIX]
        ox_kp = offv[:, 1][:, yg, xloc]
        m_kp = msk[bb][:, yg, xloc]
        by = yg[None, :] - 1 + (karr // 3)[:, None]
        bx = xloc[None, :] - 1 + (karr % 3)[:, None]

        y0 = np.floor(oy_kp)
        x0 = np.floor(ox_kp)
        ly = oy_kp - y0
        lx = ox_kp - x0
        idx_kp = (y0 + by + PAD) * TG + (x0 + bx + PAD)     # [K, NPIX]

        idxj = tojd(idx_kp).astype(np.int64)
        assert idxj.min() >= 0 and idxj.max() < NROWS
        idx16 = np.ascontiguousarray(
            idxj.reshape(NSAMP // 16, 16).T).astype(np.int16)

        mj = tojd(m_kp).astype(f32)
        lxj = tojd(lx).astype(f32)
        lyj = tojd(ly).astype(f32)
        # corner weights matching planes P00..P11, [4, NSAMP]
        bws = np.stack([mj * (1 - lyj) * (1 - lxj),
                        mj * (1 - lyj) * lxj,
                        mj * lyj * (1 - lxj),
                        mj * lyj * lxj], 0)
        # -> [128 px, NBLK, NPC, K, 4]
        bw_host = np.ascontiguousarray(
            bws.reshape(4, NBLK, NPC, K, PCW).transpose(4, 1, 2, 3, 0)
        ).astype(BF16)

        in_maps.append({
            "tab": tabs[bb],
            "idx": idx16,
            "bw": bw_host,
            "wt": wt.astype(BF16),
            "idn": idn,
            "bias": bias2,
        })
    return in_maps


def _install_ntff_shim():
    """Provide antenv.axon_hooks (missing in this image) so trace=True works."""
    import types
    if "antenv.axon_hooks" in sys.modules:
        return
    sys.path.insert(0, "/root/.axon_site")
    from trn_agent_boot.trn_boot import _ntff_profile_via_ctypes
    hook = _ntff_profile_via_ctypes("/opt/axon/libaxon_pjrt.so")
    mod = types.ModuleType("antenv.axon_hooks")
    mod.get_axon_ntff_profile_hook = lambda: hook
    mod.set_axon_ntff_profile_hook = lambda h: None
    sys.modules["antenv.axon_hooks"] = mod


def kernel(input, offset, mask, weight, bias, _trace=False):
    if _trace:
        _install_ntff_shim()
    from concourse.bass_utils import run_bass_kernel_spmd

    if "nc" not in _CACHE:
        _CACHE["nc"] = _build_program()
    nc = _CACHE["nc"]

    in_maps = _host_prep(input, offset, mask, weight, bias)
    res = run_bass_kernel_spmd(
        nc, in_maps, core_ids=list(range(8)),
        trace=_trace,
        trace_cores=list(range(8)) if _trace else None,
    )
    kernel.last_results = res

    out = np.empty((B, O, H, W), np.float32)
    for core in range(8):
        bb, h = divmod(core, 2)
        blockout = res.results[core]["out"]       # [64, 8192] f32
        out[bb, :, h * 64:(h + 1) * 64, :] = blockout.reshape(O, 64, W)
    return out
